# revision 22
# baseline (speedup 1.0000x reference)
"""Bahdanau attention Trainium2 kernel.

reference (per batch b):
  pq = queries[b] @ W                     # [1, A]
  pi = items[b] @ U                       # [S, A]
  added = tanh(pq + pi)                   # [S, A]
  r = added @ v                           # [S]
  r = where(weights==0, -inf, r)
  scores = softmax(r)                     # [S]
  blended = scores @ items[b]             # [I]

Sharding: data-parallel over batch B=32 across 8 cores (4 batches/core,
no collectives).

Per-core dataflow (hot matmuls in float32r = fp22, full PE rate):
  - X chunk [512, 1024] loaded naturally, PE-transposed (is_transpose)
    into XT [i-part, s-free].
  - added^T[a,s] accumulated in PSUM over 8 i-chunks:
      matmul(lhsT=U[i,a-chunk], rhs=XT[i, s-chunk])
    tanh applied on ACT with per-partition bias pq^T[a, b].
  - reactivity row [1, 512] = sum_a v[a] * added^T[a, s] via 8
    accumulating matmuls (lhsT = v chunk [128, 1]).
  - softmax without max-subtraction (reactivity ~ N(0,1), exp safe):
    p = exp(r) * mask, Z = sum(p); no rescaling so the blend matmul
    accumulates over the whole batch directly in PSUM.
  - row->column conversions done on the PE with K=1 matmuls
    (lhsT = row slice [1,128], rhs = [[1.0]]).
  - PE emission is software-pipelined: chunk k+1's X transposes are
    interleaved between chunk k's pi-matmul groups, v-matmuls trail the
    pi-matmuls by one a-chunk, and the reactivity->blend tail of chunk
    k is emitted inside chunk k+1 so the PE rarely waits on ACT/DVE.
  - U and W stream on the SWDGE (gpsimd) DMA path so they overlap the
    HWDGE X loads during startup; W is consumed in two halves.
"""

import sys

sys.path.insert(0, "/opt/trn_rl_repo")

import numpy as np

B, S, Q, I, A = 32, 2048, 1024, 1024, 1024
NCORES = 8
BPC = B // NCORES  # batches per core
SC = 512  # s-chunk size
NSC = S // SC  # 4 chunks per batch
NCH = BPC * NSC  # total chunks per core
NIC = I // 128  # 8 i-chunks
NAC = A // 128  # 8 a-chunks

_cache: dict = {}


def _build():
    import concourse.mybir as mybir
    import concourse.tile as tile
    from concourse import bacc, bass_isa
    from concourse.masks import make_identity

    dt = mybir.dt
    f32, f32r, i32 = dt.float32, dt.float32r, dt.int32
    AF = mybir.ActivationFunctionType
    OP = mybir.AluOpType
    AX = mybir.AxisListType

    nc = bacc.Bacc("TRN2", target_bir_lowering=False, debug=False)

    queries_h = nc.dram_tensor("queries", [BPC, 1, Q], f32, kind="ExternalInput")
    items_h = nc.dram_tensor("multiple_items", [BPC, S, I], f32, kind="ExternalInput")
    weights_h = nc.dram_tensor("weights", [BPC, S], i32, kind="ExternalInput")
    W_h = nc.dram_tensor("W", [Q, A], f32, kind="ExternalInput")
    U_h = nc.dram_tensor("U", [I, A], f32, kind="ExternalInput")
    v_h = nc.dram_tensor("v", [A], f32, kind="ExternalInput")
    blended_h = nc.dram_tensor("blended", [BPC, I], f32, kind="ExternalOutput")
    scores_h = nc.dram_tensor("scores", [BPC, S], f32, kind="ExternalOutput")

    queries = queries_h.ap()
    items = items_h.ap()
    weights = weights_h.ap()
    Wap = W_h.ap()
    Uap = U_h.ap()
    vap = v_h.ap()
    blended = blended_h.ap()
    scores = scores_h.ap()

    def chunk_bs(k):
        return k // NSC, k % NSC

    with tile.TileContext(nc) as tc:
        with (
            tc.tile_pool(name="persist", bufs=1) as persist,
            tc.tile_pool(name="small", bufs=4) as small,
            tc.tile_pool(name="xpool", bufs=3) as xpool,
            tc.tile_pool(name="xtpool", bufs=2) as xtpool,
            tc.tile_pool(name="addpool", bufs=3) as addpool,
            tc.tile_pool(name="ppool", bufs=2) as ppool,
            tc.tile_pool(name="opool", bufs=2) as opool,
            tc.tile_pool(name="setup", bufs=1) as setup,
            tc.tile_pool(name="setup_w", bufs=1) as setup_w,
            tc.tile_pool(name="xtpsum", bufs=2, space="PSUM") as xtpsum,
            tc.tile_pool(name="pipsum", bufs=2, space="PSUM") as pipsum,
            tc.tile_pool(name="rpsum", bufs=1, space="PSUM") as rpsum,
            tc.tile_pool(name="rcpsum", bufs=1, space="PSUM") as rcpsum,
            tc.tile_pool(name="blpsum", bufs=2, space="PSUM") as blpsum,
        ):
            # ---- persistent constants (cheap, no big DMA deps) ----
            ident_f = persist.tile([128, 128], f32)
            make_identity(nc, ident_f)
            ident_r = persist.tile([128, 128], f32r)
            nc.vector.tensor_copy(out=ident_r, in_=ident_f)
            one_f = persist.tile([1, 1], f32)
            nc.vector.memset(one_f, 1.0)
            ident4 = persist.tile([BPC, BPC], f32)
            make_identity(nc, ident4)

            v_sb = persist.tile([128, NAC], f32r)
            pqT_sb = persist.tile([128, NAC, BPC], f32)
            m_col = persist.tile([128, S // 128, BPC], f32)

            # ---- HWDGE queue: small loads, then X(0), X(1) ----
            v_row = setup.tile([1, A], f32)
            nc.sync.dma_start(out=v_row, in_=vap[None, :])
            q_sb = setup.tile([BPC, Q], f32)
            nc.sync.dma_start(out=q_sb, in_=queries[:, 0, :])

            def load_x(k):
                b, sc = chunk_bs(k)
                x_sb = xpool.tile([128, SC // 128, I], f32r, tag="x", name="x_sb")
                nc.sync.dma_start(
                    out=x_sb,
                    in_=items[b, sc * SC : (sc + 1) * SC, :]
                    .rearrange("(c p) i -> p c i", p=128)
                    .bitcast(f32r),
                )
                return x_sb

            x_tiles = {0: load_x(0), 1: load_x(1)}

            # ---- SWDGE queue: W halves then U (overlaps HWDGE X loads) ----
            w_half = {}
            for h in range(2):
                w_half[h] = setup_w.tile(
                    [128, Q // 128, 512], f32r, tag=f"wsb{h}", name=f"w_half{h}"
                )
                nc.gpsimd.dma_start(
                    out=w_half[h],
                    in_=Wap.rearrange("(c p) a -> p c a", p=128)[
                        :, :, h * 512 : (h + 1) * 512
                    ].bitcast(f32r),
                )
            u_sb = persist.tile([128, NIC, A], f32r)
            u_re = Uap.rearrange("(c p) a -> p c a", p=128).bitcast(f32r)
            for ic in range(NIC):
                nc.gpsimd.dma_start(out=u_sb[:, ic, :], in_=u_re[:, ic, :])

            # ---- setup PE work with only small-DMA deps ----
            psum_v = rcpsum.tile([128, NAC], f32, tag="rc", name="psum_v")
            for c in range(NAC):
                nc.tensor.matmul(
                    psum_v[:, c : c + 1],
                    lhsT=v_row[:, c * 128 : (c + 1) * 128],
                    rhs=one_f,
                    start=True,
                    stop=True,
                )
            nc.vector.tensor_copy(out=v_sb, in_=psum_v)

            psum_qT = rcpsum.tile([128, Q // 128, BPC], f32, tag="rc", name="psum_qT")
            for c in range(Q // 128):
                nc.tensor.matmul(
                    psum_qT[:, c, :],
                    lhsT=q_sb[:, c * 128 : (c + 1) * 128],
                    rhs=ident4,
                    start=True,
                    stop=True,
                )
            qT_sb = setup.tile([128, Q // 128, BPC], f32r)
            nc.vector.tensor_copy(out=qT_sb, in_=psum_qT)

            # pq accumulators (written in two half-phases inside chunk 0)
            pq_sb = setup.tile([BPC, A], f32)

            def emit_pq_half(h):
                psum_pq = rcpsum.tile([BPC, 512], f32, tag="rc", name="psum_pq")
                for qc in range(Q // 128):
                    nc.tensor.matmul(
                        psum_pq,
                        lhsT=qT_sb[:, qc, :],
                        rhs=w_half[h][:, qc, :],
                        start=(qc == 0),
                        stop=(qc == Q // 128 - 1),
                    )
                nc.vector.tensor_copy(out=pq_sb[:, h * 512 : (h + 1) * 512], in_=psum_pq)

            def emit_pqT():
                psum_pqT = rcpsum.tile([128, NAC, BPC], f32, tag="rc", name="psum_pqT")
                for c in range(NAC):
                    nc.tensor.matmul(
                        psum_pqT[:, c, :],
                        lhsT=pq_sb[:, c * 128 : (c + 1) * 128],
                        rhs=ident4,
                        start=True,
                        stop=True,
                    )
                nc.vector.tensor_copy(out=pqT_sb, in_=psum_pqT)

            def emit_masks(b):
                """Mask columnar for batch b: m_col[:, cc, b] = f32(weights[b])"""
                w_row_i = setup.tile([1, S], i32, tag="wrow_i", name="w_row_i")
                nc.sync.dma_start(out=w_row_i, in_=weights[b : b + 1, :])
                w_row_f = setup.tile([1, S], f32, tag="wrow_f", name="w_row_f")
                nc.vector.tensor_copy(out=w_row_f, in_=w_row_i)
                psum_m = rcpsum.tile([128, S // 128], f32, tag="rc", name="psum_m")
                for c in range(S // 128):
                    nc.tensor.matmul(
                        psum_m[:, c : c + 1],
                        lhsT=w_row_f[:, c * 128 : (c + 1) * 128],
                        rhs=one_f,
                        start=True,
                        stop=True,
                    )
                nc.vector.tensor_copy(out=m_col[:, :, b], in_=psum_m)

            # ---- main pipeline helpers ----
            def emit_tr_group(k, gid, xt_tiles, eng=None):
                """Four PE transposes + one PSUM->SBUF copy for chunk k."""
                c, g = gid // 2, gid % 2
                x_sb = x_tiles[k]
                if k not in xt_tiles:
                    xt_tiles[k] = xtpool.tile(
                        [128, NIC, SC], f32r, tag="xt", name="xt_sb"
                    )
                xt_sb = xt_tiles[k]
                psum_xt = xtpsum.tile([128, 4, 128], f32r, tag="xtp", name="psum_xt")
                for j in range(4):
                    ic = 4 * g + j
                    nc.tensor.transpose(
                        psum_xt[:, j, :],
                        x_sb[:, c, ic * 128 : (ic + 1) * 128],
                        ident_r,
                    )
                dst = xt_sb[:, 4 * g : 4 * g + 4, c * 128 : (c + 1) * 128]
                if eng is None and gid % 2 == 1:
                    nc.scalar.copy(out=dst, in_=psum_xt)
                else:
                    nc.vector.tensor_copy(out=dst, in_=psum_xt)

            def emit_epilogue(b, p_col, psum_bl):
                """Z, 1/Z, blended + scores outputs for batch b."""
                z_col = small.tile([128, 1], f32, tag="z_col", name="z_col")
                nc.vector.tensor_reduce(z_col, p_col, axis=AX.X, op=OP.add)
                z_all = small.tile([128, 1], f32, tag="z_all", name="z_all")
                nc.gpsimd.partition_all_reduce(
                    z_all, z_col, channels=128, reduce_op=bass_isa.ReduceOp.add
                )
                rz_col = small.tile([128, 1], f32, tag="rz_col", name="rz_col")
                nc.vector.reciprocal(rz_col, z_all)

                blended_sb = opool.tile([1, I], f32, tag="blended", name="blended_sb")
                for h in range(2):
                    nc.vector.tensor_scalar_mul(
                        blended_sb[:, h * 512 : (h + 1) * 512],
                        psum_bl[h],
                        rz_col[:1, :],
                    )
                nc.sync.dma_start(out=blended[b : b + 1, :], in_=blended_sb)

                p_scaled = opool.tile(
                    [128, S // 128], f32, tag="p_scaled", name="p_scaled"
                )
                nc.vector.tensor_scalar_mul(p_scaled, p_col, rz_col)
                psum_srow = xtpsum.tile([S // 128, 128], f32, tag="xtp", name="psum_srow")
                nc.tensor.transpose(psum_srow, p_scaled, ident_f)
                s_row = opool.tile([S // 128, 128], f32, tag="s_row", name="s_row")
                nc.vector.tensor_copy(out=s_row, in_=psum_srow)
                nc.sync.dma_start(
                    out=scores[b].rearrange("(c p) -> c p", p=128), in_=s_row
                )

            def emit_tail_rtr(t):
                """reactivity row -> columnar -> p_col for a finished chunk."""
                b, sc = t["b"], t["sc"]
                r_row = small.tile([1, SC], f32, tag="r_row", name="r_row")
                nc.vector.tensor_copy(out=r_row, in_=t["psum_r"])
                psum_rc = rcpsum.tile([128, SC // 128], f32, tag="rc", name="psum_rc")
                for c in range(SC // 128):
                    nc.tensor.matmul(
                        psum_rc[:, c : c + 1],
                        lhsT=r_row[:, c * 128 : (c + 1) * 128],
                        rhs=one_f,
                        start=True,
                        stop=True,
                    )
                p_raw = small.tile([128, SC // 128], f32, tag="p_raw", name="p_raw")
                nc.scalar.activation(out=p_raw, in_=psum_rc, func=AF.Exp)
                nc.vector.tensor_tensor(
                    t["p_col"][:, sc * 4 : sc * 4 + 4],
                    p_raw,
                    m_col[:, sc * 4 : sc * 4 + 4, b],
                    OP.mult,
                )

            def emit_tail_blend(t):
                """blend matmuls for a finished chunk (+ epilogue at batch end)."""
                b, sc = t["b"], t["sc"]
                for c in range(SC // 128):
                    for h in range(2):
                        nc.tensor.matmul(
                            t["psum_bl"][h],
                            lhsT=t["p_col"][:, sc * 4 + c : sc * 4 + c + 1],
                            rhs=t["x_sb"][:, c, h * 512 : (h + 1) * 512],
                            start=(sc == 0 and c == 0),
                            stop=(sc == NSC - 1 and c == SC // 128 - 1),
                        )
                if sc == NSC - 1:
                    emit_epilogue(b, t["p_col"], t["psum_bl"])

            def emit_vmm(psum_r, ac, added, start, stop):
                nc.tensor.matmul(
                    psum_r,
                    lhsT=v_sb[:, ac : ac + 1],
                    rhs=added,
                    start=start,
                    stop=stop,
                )

            # ---- main loop ----
            xt_tiles: dict = {}
            batch_state: dict = {}
            tail = None
            for k in range(NCH):
                b, sc = chunk_bs(k)
                if sc == 0:
                    emit_masks(b)
                    p_col = ppool.tile([128, S // 128], f32r, tag="p_col", name="p_col")
                    psum_bl = [
                        blpsum.tile([1, 512], f32, tag="bl", name=f"bl{h}")
                        for h in range(2)
                    ]
                    batch_state[b] = (p_col, psum_bl)
                p_col, psum_bl = batch_state[b]

                if k >= 1 and k + 1 < NCH:
                    x_tiles[k + 1] = load_x(k + 1)

                if k == 0:
                    # prologue: transposes for chunk 0, then pq/pqT (gated on
                    # the SWDGE W halves) so tanh's bias is ready before the
                    # first tanh is emitted
                    for gid in range(8):
                        emit_tr_group(0, gid, xt_tiles)
                    emit_pq_half(0)
                    emit_pq_half(1)
                    emit_pqT()

                psum_r = rpsum.tile([1, SC], f32, tag="r", name="psum_r")
                added_tiles = []
                for ac in range(NAC):
                    psum_pi = pipsum.tile([128, SC], f32, tag="pi", name="psum_pi")
                    for ic in range(NIC):
                        nc.tensor.matmul(
                            psum_pi,
                            lhsT=u_sb[:, ic, ac * 128 : (ac + 1) * 128],
                            rhs=xt_tiles[k][:, ic, :],
                            start=(ic == 0),
                            stop=(ic == NIC - 1),
                        )
                    added = addpool.tile([128, SC], f32r, tag="added", name="added")
                    nc.scalar.activation(
                        out=added,
                        in_=psum_pi,
                        func=AF.Tanh,
                        bias=pqT_sb[:, ac, b : b + 1],
                    )
                    added_tiles.append(added)

                    # previous chunk's last v-matmul, tucked after mmg(0) so
                    # the PE never waits on its tanh
                    if ac == 0 and tail is not None:
                        emit_vmm(
                            tail["psum_r"], NAC - 1, tail["added"][NAC - 1], False, True
                        )
                    if ac == 1 and tail is not None:
                        emit_tail_rtr(tail)
                    if ac == 2 and tail is not None:
                        emit_tail_blend(tail)
                    if 4 <= ac <= 7 and k + 1 < NCH:
                        emit_tr_group(k + 1, 2 * (ac - 4), xt_tiles)
                        emit_tr_group(k + 1, 2 * (ac - 4) + 1, xt_tiles)
                    if ac >= 1:
                        emit_vmm(psum_r, ac - 1, added_tiles[ac - 1], ac == 1, False)

                tail = {
                    "b": b,
                    "sc": sc,
                    "psum_r": psum_r,
                    "added": added_tiles,
                    "x_sb": x_tiles[k],
                    "p_col": p_col,
                    "psum_bl": psum_bl,
                }

            # drain the final chunk's tail
            emit_vmm(tail["psum_r"], NAC - 1, tail["added"][NAC - 1], False, True)
            emit_tail_rtr(tail)
            emit_tail_blend(tail)

    nc.compile()
    return nc


def _get_nc():
    if "nc" not in _cache:
        _cache["nc"] = _build()
    return _cache["nc"]


def kernel(queries, multiple_items, weights, W, U, v):
    from concourse import bass_utils

    nc = _get_nc()

    queries = np.ascontiguousarray(np.asarray(queries, dtype=np.float32))
    multiple_items = np.ascontiguousarray(np.asarray(multiple_items, dtype=np.float32))
    weights = np.ascontiguousarray(np.asarray(weights, dtype=np.int32))
    W = np.ascontiguousarray(np.asarray(W, dtype=np.float32))
    U = np.ascontiguousarray(np.asarray(U, dtype=np.float32))
    v = np.ascontiguousarray(np.asarray(v, dtype=np.float32))

    in_maps = []
    for c in range(NCORES):
        sl = slice(c * BPC, (c + 1) * BPC)
        in_maps.append(
            {
                "queries": queries[sl],
                "multiple_items": multiple_items[sl],
                "weights": weights[sl],
                "W": W,
                "U": U,
                "v": v,
            }
        )

    res = bass_utils.run_bass_kernel_spmd(nc, in_maps, core_ids=list(range(NCORES)))
    blended = np.concatenate([res.results[c]["blended"] for c in range(NCORES)], axis=0)
    scores = np.concatenate([res.results[c]["scores"] for c in range(NCORES)], axis=0)
    return blended, scores


# revision 31
# speedup vs baseline: 1.0101x; 1.0101x over previous
"""Bahdanau attention Trainium2 kernel.

reference (per batch b):
  pq = queries[b] @ W                     # [1, A]
  pi = items[b] @ U                       # [S, A]
  added = tanh(pq + pi)                   # [S, A]
  r = added @ v                           # [S]
  r = where(weights==0, -inf, r)
  scores = softmax(r)                     # [S]
  blended = scores @ items[b]             # [I]

Sharding: data-parallel over batch B=32 across 8 cores (4 batches/core,
no collectives).

Per-core dataflow (hot matmuls in float32r = fp22, full PE rate):
  - X chunk [512, 1024] loaded naturally, PE-transposed (is_transpose)
    into XT [i-part, s-free].
  - added^T[a,s] accumulated in PSUM over 8 i-chunks:
      matmul(lhsT=U[i,a-chunk], rhs=XT[i, s-chunk])
    tanh applied on ACT with per-partition bias pq^T[a, b].
  - reactivity row [1, 512] = sum_a v[a] * added^T[a, s] via 8
    accumulating matmuls (lhsT = v chunk [128, 1]).
  - softmax without max-subtraction (reactivity ~ N(0,1), exp safe):
    p = exp(r) * mask, Z = sum(p); no rescaling so the blend matmul
    accumulates over the whole batch directly in PSUM.
  - row->column conversions done on the PE with K=1 matmuls
    (lhsT = row slice [1,128], rhs = [[1.0]]).
  - PE emission is software-pipelined: chunk k+1's X transposes are
    interleaved between chunk k's pi-matmul groups, v-matmuls trail the
    pi-matmuls by one a-chunk, and the reactivity->blend tail of chunk
    k is emitted inside chunk k+1 so the PE rarely waits on ACT/DVE.
  - U and W stream on the SWDGE (gpsimd) DMA path so they overlap the
    HWDGE X loads during startup; W is consumed in two halves.
"""

import sys

sys.path.insert(0, "/opt/trn_rl_repo")

import numpy as np

B, S, Q, I, A = 32, 2048, 1024, 1024, 1024
NCORES = 8
BPC = B // NCORES  # batches per core
SC = 512  # s-chunk size
NSC = S // SC  # 4 chunks per batch
NCH = BPC * NSC  # total chunks per core
NIC = I // 128  # 8 i-chunks
NAC = A // 128  # 8 a-chunks

_cache: dict = {}


def _build():
    import concourse.mybir as mybir
    import concourse.tile as tile
    from concourse import bacc, bass_isa
    from concourse.masks import make_identity

    dt = mybir.dt
    f32, f32r, i32 = dt.float32, dt.float32r, dt.int32
    AF = mybir.ActivationFunctionType
    OP = mybir.AluOpType
    AX = mybir.AxisListType

    nc = bacc.Bacc("TRN2", target_bir_lowering=False, debug=False)

    queries_h = nc.dram_tensor("queries", [BPC, 1, Q], f32, kind="ExternalInput")
    items_h = nc.dram_tensor("multiple_items", [BPC, S, I], f32, kind="ExternalInput")
    weights_h = nc.dram_tensor("weights", [BPC, S], i32, kind="ExternalInput")
    W_h = nc.dram_tensor("W", [Q, A], f32, kind="ExternalInput")
    U_h = nc.dram_tensor("U", [I, A], f32, kind="ExternalInput")
    v_h = nc.dram_tensor("v", [A], f32, kind="ExternalInput")
    blended_h = nc.dram_tensor("blended", [BPC, I], f32, kind="ExternalOutput")
    scores_h = nc.dram_tensor("scores", [BPC, S], f32, kind="ExternalOutput")

    queries = queries_h.ap()
    items = items_h.ap()
    weights = weights_h.ap()
    Wap = W_h.ap()
    Uap = U_h.ap()
    vap = v_h.ap()
    blended = blended_h.ap()
    scores = scores_h.ap()

    def chunk_bs(k):
        return k // NSC, k % NSC

    with tile.TileContext(nc) as tc:
        with (
            tc.tile_pool(name="persist", bufs=1) as persist,
            tc.tile_pool(name="small", bufs=4) as small,
            tc.tile_pool(name="xpool", bufs=3) as xpool,
            tc.tile_pool(name="xtpool", bufs=2) as xtpool,
            tc.tile_pool(name="addpool", bufs=3) as addpool,
            tc.tile_pool(name="ppool", bufs=2) as ppool,
            tc.tile_pool(name="opool", bufs=2) as opool,
            tc.tile_pool(name="setup", bufs=1) as setup,
            tc.tile_pool(name="setup_w", bufs=1) as setup_w,
            tc.tile_pool(name="xtpsum", bufs=2, space="PSUM") as xtpsum,
            tc.tile_pool(name="pipsum", bufs=2, space="PSUM") as pipsum,
            tc.tile_pool(name="rpsum", bufs=1, space="PSUM") as rpsum,
            tc.tile_pool(name="rcpsum", bufs=1, space="PSUM") as rcpsum,
            tc.tile_pool(name="blpsum", bufs=2, space="PSUM") as blpsum,
        ):
            # ---- persistent constants (cheap, no big DMA deps) ----
            ident_f = persist.tile([128, 128], f32)
            make_identity(nc, ident_f)
            ident_r = persist.tile([128, 128], f32r)
            nc.vector.tensor_copy(out=ident_r, in_=ident_f)
            one_f = persist.tile([1, 1], f32)
            nc.vector.memset(one_f, 1.0)
            ident4 = persist.tile([BPC, BPC], f32)
            make_identity(nc, ident4)

            v_sb = persist.tile([128, NAC], f32r)
            pqT_sb = persist.tile([128, NAC, BPC], f32)

            # ---- HWDGE queue: small loads, then X(0), X(1) ----
            v_row = setup.tile([1, A], f32)
            nc.sync.dma_start(out=v_row, in_=vap[None, :])
            q_sb = setup.tile([BPC, Q], f32)
            nc.sync.dma_start(out=q_sb, in_=queries[:, 0, :])

            def load_x(k):
                b, sc = chunk_bs(k)
                x_sb = xpool.tile([128, SC // 128, I], f32r, tag="x", name="x_sb")
                nc.sync.dma_start(
                    out=x_sb,
                    in_=items[b, sc * SC : (sc + 1) * SC, :]
                    .rearrange("(c p) i -> p c i", p=128)
                    .bitcast(f32r),
                )
                return x_sb

            x0 = xpool.tile([128, SC // 128, I], f32r, tag="x", name="x_sb")
            items_r = items.bitcast(f32r)
            for c in range(SC // 128):
                nc.sync.dma_start(
                    out=x0[:, c, :],
                    in_=items_r[0, c * 128 : (c + 1) * 128, :].rearrange(
                        "p i -> p i"
                    ),
                )
            x_tiles = {0: x0, 1: load_x(1)}

            # ---- SWDGE queue: W halves then U (overlaps HWDGE X loads) ----
            w_half = {}
            for h in range(2):
                w_half[h] = setup_w.tile(
                    [128, Q // 128, 512], f32r, tag=f"wsb{h}", name=f"w_half{h}"
                )
                nc.gpsimd.dma_start(
                    out=w_half[h],
                    in_=Wap.rearrange("(c p) a -> p c a", p=128)[
                        :, :, h * 512 : (h + 1) * 512
                    ].bitcast(f32r),
                )
            u_sb = persist.tile([128, NIC, A], f32r)
            u_re = Uap.rearrange("(c p) a -> p c a", p=128).bitcast(f32r)
            for ic in range(NIC):
                nc.gpsimd.dma_start(out=u_sb[:, ic, :], in_=u_re[:, ic, :])

            # ---- setup PE work with only small-DMA deps ----
            psum_v = rcpsum.tile([128, NAC], f32, tag="rc", name="psum_v")
            for c in range(NAC):
                nc.tensor.matmul(
                    psum_v[:, c : c + 1],
                    lhsT=v_row[:, c * 128 : (c + 1) * 128],
                    rhs=one_f,
                    start=True,
                    stop=True,
                )
            nc.vector.tensor_copy(out=v_sb, in_=psum_v)

            psum_qT = rcpsum.tile([128, Q // 128, BPC], f32, tag="rc", name="psum_qT")
            for c in range(Q // 128):
                nc.tensor.matmul(
                    psum_qT[:, c, :],
                    lhsT=q_sb[:, c * 128 : (c + 1) * 128],
                    rhs=ident4,
                    start=True,
                    stop=True,
                )
            qT_sb = setup.tile([128, Q // 128, BPC], f32r)
            nc.vector.tensor_copy(out=qT_sb, in_=psum_qT)

            # pq accumulators (written in two half-phases inside chunk 0)
            pq_sb = setup.tile([BPC, A], f32)

            def emit_pq_half(h):
                psum_pq = rcpsum.tile([BPC, 512], f32, tag="rc", name="psum_pq")
                for qc in range(Q // 128):
                    nc.tensor.matmul(
                        psum_pq,
                        lhsT=qT_sb[:, qc, :],
                        rhs=w_half[h][:, qc, :],
                        start=(qc == 0),
                        stop=(qc == Q // 128 - 1),
                    )
                nc.vector.tensor_copy(out=pq_sb[:, h * 512 : (h + 1) * 512], in_=psum_pq)

            def emit_pqT():
                psum_pqT = rcpsum.tile([128, NAC, BPC], f32, tag="rc", name="psum_pqT")
                for c in range(NAC):
                    nc.tensor.matmul(
                        psum_pqT[:, c, :],
                        lhsT=pq_sb[:, c * 128 : (c + 1) * 128],
                        rhs=ident4,
                        start=True,
                        stop=True,
                    )
                nc.vector.tensor_copy(out=pqT_sb, in_=psum_pqT)

            def emit_masks(b):
                """Mask row for batch b: w_row_f = f32(weights[b]) in {0.0, 1.0}"""
                w_row_i = setup.tile([1, S], i32, tag="wrow_i", name="w_row_i")
                nc.sync.dma_start(out=w_row_i, in_=weights[b : b + 1, :])
                w_row_f = setup.tile([1, S], f32, tag="wrow_f", name="w_row_f")
                nc.vector.tensor_copy(out=w_row_f, in_=w_row_i)
                return w_row_f

            # ---- main pipeline helpers ----
            def emit_tr_group(k, gid, xt_tiles, eng=None):
                """Four PE transposes + one PSUM->SBUF copy for chunk k."""
                c, g = gid // 2, gid % 2
                x_sb = x_tiles[k]
                if k not in xt_tiles:
                    xt_tiles[k] = xtpool.tile(
                        [128, NIC, SC], f32r, tag="xt", name="xt_sb"
                    )
                xt_sb = xt_tiles[k]
                psum_xt = xtpsum.tile([128, 4, 128], f32r, tag="xtp", name="psum_xt")
                for j in range(4):
                    ic = 4 * g + j
                    nc.tensor.transpose(
                        psum_xt[:, j, :],
                        x_sb[:, c, ic * 128 : (ic + 1) * 128],
                        ident_r,
                    )
                dst = xt_sb[:, 4 * g : 4 * g + 4, c * 128 : (c + 1) * 128]
                if eng is None and gid % 2 == 1:
                    nc.scalar.copy(out=dst, in_=psum_xt)
                else:
                    nc.vector.tensor_copy(out=dst, in_=psum_xt)

            def emit_epilogue(b, p_col, psum_bl):
                """Z, 1/Z, blended + scores outputs for batch b."""
                z_col = small.tile([128, 1], f32, tag="z_col", name="z_col")
                nc.vector.tensor_reduce(z_col, p_col, axis=AX.X, op=OP.add)
                z_all = small.tile([128, 1], f32, tag="z_all", name="z_all")
                nc.gpsimd.partition_all_reduce(
                    z_all, z_col, channels=128, reduce_op=bass_isa.ReduceOp.add
                )
                rz_col = small.tile([128, 1], f32, tag="rz_col", name="rz_col")
                nc.vector.reciprocal(rz_col, z_all)

                blended_sb = opool.tile([1, I], f32, tag="blended", name="blended_sb")
                for h in range(2):
                    nc.vector.tensor_scalar_mul(
                        blended_sb[:, h * 512 : (h + 1) * 512],
                        psum_bl[h],
                        rz_col[:1, :],
                    )
                nc.sync.dma_start(out=blended[b : b + 1, :], in_=blended_sb)

                p_scaled = opool.tile(
                    [128, S // 128], f32, tag="p_scaled", name="p_scaled"
                )
                nc.vector.tensor_scalar_mul(p_scaled, p_col, rz_col)
                psum_srow = xtpsum.tile([S // 128, 128], f32, tag="xtp", name="psum_srow")
                nc.tensor.transpose(psum_srow, p_scaled, ident_f)
                s_row = opool.tile([S // 128, 128], f32, tag="s_row", name="s_row")
                nc.vector.tensor_copy(out=s_row, in_=psum_srow)
                nc.sync.dma_start(
                    out=scores[b].rearrange("(c p) -> c p", p=128), in_=s_row
                )

            def emit_tail_r1(t):
                """p row = exp(reactivity) * mask for a finished chunk."""
                sc = t["sc"]
                p_exp = small.tile([1, SC], f32, tag="p_exp", name="p_exp")
                nc.scalar.activation(out=p_exp, in_=t["psum_r"], func=AF.Exp)
                p_row = small.tile([1, SC], f32, tag="p_row", name="p_row")
                nc.vector.tensor_tensor(
                    p_row,
                    p_exp,
                    t["w_row_f"][:, sc * SC : (sc + 1) * SC],
                    OP.mult,
                )
                t["p_row"] = p_row

            def emit_tail_r2(t):
                """p row -> columnar p_col slice (PE K=1 matmuls)."""
                sc = t["sc"]
                p_row = t["p_row"]
                psum_rc = rcpsum.tile([128, SC // 128], f32, tag="rc", name="psum_rc")
                for c in range(SC // 128):
                    nc.tensor.matmul(
                        psum_rc[:, c : c + 1],
                        lhsT=p_row[:, c * 128 : (c + 1) * 128],
                        rhs=one_f,
                        start=True,
                        stop=True,
                    )
                nc.vector.tensor_copy(
                    out=t["p_col"][:, sc * 4 : sc * 4 + 4], in_=psum_rc
                )

            def emit_tail_blend(t):
                """blend matmuls for a finished chunk (+ epilogue at batch end)."""
                b, sc = t["b"], t["sc"]
                for c in range(SC // 128):
                    for h in range(2):
                        nc.tensor.matmul(
                            t["psum_bl"][h],
                            lhsT=t["p_col"][:, sc * 4 + c : sc * 4 + c + 1],
                            rhs=t["x_sb"][:, c, h * 512 : (h + 1) * 512],
                            start=(sc == 0 and c == 0),
                            stop=(sc == NSC - 1 and c == SC // 128 - 1),
                        )
                if sc == NSC - 1:
                    emit_epilogue(b, t["p_col"], t["psum_bl"])

            def emit_vmm(psum_r, ac, added, start, stop):
                nc.tensor.matmul(
                    psum_r,
                    lhsT=v_sb[:, ac : ac + 1],
                    rhs=added,
                    start=start,
                    stop=stop,
                )

            # ---- main loop ----
            xt_tiles: dict = {}
            batch_state: dict = {}
            tail = None
            for k in range(NCH):
                b, sc = chunk_bs(k)
                if sc == 0:
                    w_row_f = emit_masks(b)
                    p_col = ppool.tile([128, S // 128], f32r, tag="p_col", name="p_col")
                    psum_bl = [
                        blpsum.tile([1, 512], f32, tag="bl", name=f"bl{h}")
                        for h in range(2)
                    ]
                    batch_state[b] = (p_col, psum_bl, w_row_f)
                p_col, psum_bl, w_row_f = batch_state[b]

                if k >= 1 and k + 1 < NCH:
                    x_tiles[k + 1] = load_x(k + 1)

                if k == 0:
                    # prologue: transposes for chunk 0, then pq/pqT (gated on
                    # the SWDGE W halves) so tanh's bias is ready before the
                    # first tanh is emitted
                    for gid in range(8):
                        emit_tr_group(0, gid, xt_tiles)
                    emit_pq_half(0)
                    emit_pq_half(1)
                    emit_pqT()

                psum_r = rpsum.tile([1, SC], f32, tag="r", name="psum_r")
                added_tiles = []
                for ac in range(NAC):
                    psum_pi = pipsum.tile([128, SC], f32, tag="pi", name="psum_pi")
                    for ic in range(NIC):
                        nc.tensor.matmul(
                            psum_pi,
                            lhsT=u_sb[:, ic, ac * 128 : (ac + 1) * 128],
                            rhs=xt_tiles[k][:, ic, :],
                            start=(ic == 0),
                            stop=(ic == NIC - 1),
                        )
                    added = addpool.tile([128, SC], f32r, tag="added", name="added")
                    nc.scalar.activation(
                        out=added,
                        in_=psum_pi,
                        func=AF.Tanh,
                        bias=pqT_sb[:, ac, b : b + 1],
                    )
                    added_tiles.append(added)

                    # previous chunk's last v-matmul, tucked after mmg(0) so
                    # the PE never waits on its tanh
                    if ac == 0 and tail is not None:
                        emit_vmm(
                            tail["psum_r"], NAC - 1, tail["added"][NAC - 1], False, True
                        )
                    if ac == 1 and tail is not None:
                        emit_tail_r1(tail)
                    if ac == 2 and tail is not None:
                        emit_tail_r2(tail)
                    if ac == 3 and tail is not None:
                        emit_tail_blend(tail)
                    if 4 <= ac <= 7 and k + 1 < NCH:
                        emit_tr_group(k + 1, 2 * (ac - 4), xt_tiles)
                        emit_tr_group(k + 1, 2 * (ac - 4) + 1, xt_tiles)
                    if ac >= 1:
                        emit_vmm(psum_r, ac - 1, added_tiles[ac - 1], ac == 1, False)

                tail = {
                    "b": b,
                    "sc": sc,
                    "psum_r": psum_r,
                    "added": added_tiles,
                    "x_sb": x_tiles[k],
                    "p_col": p_col,
                    "psum_bl": psum_bl,
                    "w_row_f": w_row_f,
                }

            # drain the final chunk's tail
            emit_vmm(tail["psum_r"], NAC - 1, tail["added"][NAC - 1], False, True)
            emit_tail_r1(tail)
            emit_tail_r2(tail)
            emit_tail_blend(tail)

    nc.compile()
    return nc


def _get_nc():
    if "nc" not in _cache:
        _cache["nc"] = _build()
    return _cache["nc"]


def kernel(queries, multiple_items, weights, W, U, v):
    from concourse import bass_utils

    nc = _get_nc()

    queries = np.ascontiguousarray(np.asarray(queries, dtype=np.float32))
    multiple_items = np.ascontiguousarray(np.asarray(multiple_items, dtype=np.float32))
    weights = np.ascontiguousarray(np.asarray(weights, dtype=np.int32))
    W = np.ascontiguousarray(np.asarray(W, dtype=np.float32))
    U = np.ascontiguousarray(np.asarray(U, dtype=np.float32))
    v = np.ascontiguousarray(np.asarray(v, dtype=np.float32))

    in_maps = []
    for c in range(NCORES):
        sl = slice(c * BPC, (c + 1) * BPC)
        in_maps.append(
            {
                "queries": queries[sl],
                "multiple_items": multiple_items[sl],
                "weights": weights[sl],
                "W": W,
                "U": U,
                "v": v,
            }
        )

    res = bass_utils.run_bass_kernel_spmd(nc, in_maps, core_ids=list(range(NCORES)))
    blended = np.concatenate([res.results[c]["blended"] for c in range(NCORES)], axis=0)
    scores = np.concatenate([res.results[c]["scores"] for c in range(NCORES)], axis=0)
    return blended, scores
